# revision 1
# baseline (speedup 1.0000x reference)
"""CrossAttention kernel for 8 TRN2 NeuronCores.

Sharding (Megatron head-parallel): core c owns heads {2c, 2c+1} = output
channels [128c, 128c+128).
  - column-parallel q/k/v projections (full activations in, per-core head
    channels out)
  - full attention for the core's heads (both batch elements)
  - row-parallel out projection -> partial [4096, 1024] fp32; host sums the
    8 partials. v-bias and out-bias fold exactly into the host-side
    epilogue: out = sum_c partial_c + Wo @ bv + bo (softmax rows sum to 1).

Per-core dataflow (t = b*2048 + n, 4096 tokens):
  pass0 (per 1024-token chunk): psq/psk [128ch, 512] = W @ xT chains; the
    evictions write per-head DUPLICATED tensors qTA/qTB/kTA/kTB [128, T]
    (head data on BOTH partition halves) so each head's two score matmuls
    land on disjoint PE row-groups and run concurrently on HW.
    v [t, ch] chains -> v_aug[u] [128 j, 16 jt, 130] (cols 65h+64 are ones).
  attention block (u, h, ih): for each of 16 j-tiles:
    scores: sps[:, 0:512]   = kTh[0:64].T   @ qTh[0:64, i0:i0+512]
            sps[:, 512:1024]= kTh[64:128].T @ qTh[64:128, i0+512:i0+1024]
    exp:    pexp [128, 1024] = exp(0.125 * sps)   (ACT, PSUM->SBUF fp16)
    ctx:    cps [65, 1024] += v_aug[., jt, 65h:65h+65].T @ pexp
            (row 64 accumulates the softmax denominator r[i])
  normalize: ctxT[u][64h:64h+64, i-block] = cps[0:64] * broadcast(1/r)
  outproj: po [128t, 512] = ctxT[u][:, tt].T @ wo half; DMA PSUM->DRAM fp32.

Projection / out-proj work units are interleaved between attention j-steps
so the PE fills the slack under the ACT-bound softmax exp stream.
PSUM banks: sps0/sps1 [128,1024] (2+2), cps [65,1024] (2), pj/po [128,512]
(1+1) = 8.
"""

import numpy as np
from contextlib import ExitStack

import concourse.bass as bass
import concourse.tile as tile
from concourse import bacc, mybir
from concourse.bass_utils import run_bass_kernel_spmd

AF = mybir.ActivationFunctionType

# ---- problem constants (hardcoded per contract) ----
B, N, C, H, D = 2, 2048, 1024, 16, 64
T = B * N            # 4096 flattened tokens
CH = 128             # channels per core = 2 heads * 64
NCORES = 8
SCALE = D ** -0.5    # 0.125

# ---- tunables ----
DT = mybir.dt.float16      # on-chip matmul/storage dtype
NPDT = np.float16
TCH = 1024                 # pass0 t-chunk
NTCH = T // TCH            # 4
FT = C // 128              # 8 f-tiles (contraction tiles for projections)
IH = 1024                  # attention i-block width (exp tile width)
JT = 128                   # j tile (kv) size
NJT = N // JT              # 16


def emit(tc: tile.TileContext, aps: dict):
    nc = tc.nc
    ctx = ExitStack()
    with ctx:
        const = ctx.enter_context(tc.tile_pool(name="const", bufs=1))
        persist = ctx.enter_context(tc.tile_pool(name="persist", bufs=1))
        xpool = ctx.enter_context(tc.tile_pool(name="xpool", bufs=2))
        ppool = ctx.enter_context(tc.tile_pool(name="ppool", bufs=2))
        rpool = ctx.enter_context(tc.tile_pool(name="rpool", bufs=2))
        opool = ctx.enter_context(tc.tile_pool(name="opool", bufs=4))
        psum = ctx.enter_context(tc.tile_pool(name="psum", bufs=1, space="PSUM"))

        # ---- load weights (wq first: the q chain of chunk 0 runs first) ----
        w_sb = {name: const.tile([128, FT, CH], DT, name=name, tag=name)
                for name in ("wq", "wk", "wv")}
        wo_sb = const.tile([128, C], DT, tag="wo")
        bias_sb = const.tile([128, 2], mybir.dt.float32, tag="bias")

        def load_weights(names):
            for name in names:
                nc.sync.dma_start(w_sb[name][:],
                                  aps[name].rearrange("(f p) m -> p f m", p=128))

        # ---- persistent activations ----
        # per-head q/k with head data duplicated on both partition halves
        qk = {nm: persist.tile([128, T], DT, name=nm, tag=nm)
              for nm in ("qTA", "qTB", "kTA", "kTB")}
        # v_aug[u]: [128 j, 16 jt, 130]; cols 65h+64 are ones
        v_aug = [persist.tile([128, NJT, 130], DT, name=f"vaug{u}", tag=f"vaug{u}")
                 for u in range(B)]
        for u in range(B):
            for h in range(2):
                nc.vector.memset(v_aug[u][:, :, 65 * h + 64:65 * h + 65], 1.0)
        # ctxT[u]: [128 ch, 2048 i] normalized context^T (head B rows 64-127)
        ctxT = [persist.tile([128, N], DT, name=f"ctxT{u}", tag=f"ctxT{u}")
                for u in range(B)]

        xr = {k: aps[k].rearrange("(f p) t -> p f t", p=128) for k in ("xq", "xk", "xv")}

        # pass0 / outproj share the two 1-bank psum slots pj & po; pass0 is
        # done (chunk 3 consumed during block 3) before outproj starts
        # (block 4), so phases never contend.
        pingpong = [0]

        def small_ps(name):
            tag = ("pj", "po")[pingpong[0] % 2]
            pingpong[0] += 1
            return psum.tile([128, 512], mybir.dt.float32, name=name, tag=tag)

        # ---------------- pass0: projections for one 1024-token chunk -----
        # returns (dma_fn, [compute units]); the driver kicks the DMA a
        # block ahead of when the compute units get filled in.
        def pass0_units(tch):
            gen = []
            ts = slice(tch * TCH, (tch + 1) * TCH)
            xq_t = xpool.tile([128, FT, TCH], DT, tag="xq")
            xk_t = xpool.tile([128, FT, TCH], DT, tag="xk")
            xv_t = xpool.tile([128, FT, TCH], DT, tag="xv")

            order = ([("xq", xq_t), ("xk", xk_t), ("xv", xv_t)] if tch == 0
                     else [("xk", xk_t), ("xv", xv_t), ("xq", xq_t)])

            def u_dma():
                for nm, t in order:
                    if tch == 0:
                        # interleave weight loads so each chunk-0 chain's
                        # weight lands just before its activations
                        load_weights({"xq": ["wq"], "xk": ["wk"],
                                      "xv": ["wv"]}[nm])
                    nc.sync.dma_start(t[:], xr[nm][:, :, ts])
                if tch == 0:
                    nc.sync.dma_start(wo_sb[:], aps["wo"])

            def qk_half(x_t, wname, bcol, dA, dB, half):
                def unit():
                    hs = slice(half * 512, (half + 1) * 512)
                    gts = slice(tch * TCH + half * 512,
                                tch * TCH + half * 512 + 512)
                    pj = small_ps(f"c{tch}{wname}{half}")
                    for ft in range(FT):
                        nc.tensor.matmul(pj[:], w_sb[wname][:, ft],
                                         x_t[:, ft, hs],
                                         start=(ft == 0), stop=(ft == FT - 1))
                    nc.vector.tensor_scalar_add(
                        dA[0:64, gts], pj[0:64, :], bias_sb[0:64, bcol:bcol + 1])
                    nc.vector.tensor_scalar_add(
                        dA[64:128, gts], pj[0:64, :], bias_sb[0:64, bcol:bcol + 1])
                    nc.vector.tensor_scalar_add(
                        dB[0:64, gts], pj[64:128, :], bias_sb[64:128, bcol:bcol + 1])
                    nc.vector.tensor_scalar_add(
                        dB[64:128, gts], pj[64:128, :], bias_sb[64:128, bcol:bcol + 1])
                return unit

            def v_half(vg, t4pair):
                # one 512-col psum with two t4 sub-chains (t4pair=0 -> t4 0,1)
                def unit():
                    pv = small_ps(f"c{tch}v{vg}{t4pair}")
                    pv4 = pv[:, 0:256].rearrange("p (t4 hh d) -> p t4 hh d",
                                                 t4=2, d=64)
                    for t4i in range(2):
                        t4 = t4pair * 2 + t4i
                        cs = slice(t4i * 128, (t4i + 1) * 128)
                        gcs = slice(vg * 512 + t4 * 128, vg * 512 + t4 * 128 + 128)
                        for ft in range(FT):
                            nc.tensor.matmul(pv[:, cs], xv_t[:, ft, gcs],
                                             w_sb["wv"][:, ft],
                                             start=(ft == 0), stop=(ft == FT - 1))
                    tt0 = tch * 8 + vg * 4 + t4pair * 2
                    u, jt0 = tt0 // 16, tt0 % 16
                    for h in range(2):
                        nc.vector.tensor_copy(
                            v_aug[u][:, jt0:jt0 + 2, 65 * h:65 * h + 64],
                            pv4[:, :, h])
                return unit

            q_units = [qk_half(xq_t, "wq", 0, qk["qTA"], qk["qTB"], hf)
                       for hf in range(2)]
            k_units = [qk_half(xk_t, "wk", 1, qk["kTA"], qk["kTB"], hf)
                       for hf in range(2)]
            v_units = [v_half(vg, tp) for vg in range(2) for tp in range(2)]
            if tch == 0:
                gen.extend(q_units + k_units + v_units)
            else:
                # k/v first: later j-steps of in-flight blocks need them
                gen.extend(k_units + v_units + q_units)
            return u_dma, gen

        # ---------------- out-proj for one 128-token tile ------------------
        def outproj_unit(u, tt):
            def unit():
                osb = opool.tile([128, C], DT, tag="osb")
                for oc in range(2):
                    po = small_ps(f"o{u}t{tt}{oc}")
                    nc.tensor.matmul(
                        po[:], ctxT[u][:, tt * 128:(tt + 1) * 128],
                        wo_sb[:, oc * 512:(oc + 1) * 512],
                        start=True, stop=True)
                    nc.vector.tensor_copy(osb[:, oc * 512:(oc + 1) * 512], po[:])
                nc.sync.dma_start(
                    aps["out"][u * N + tt * 128:u * N + (tt + 1) * 128, :],
                    osb[:])
            return unit

        # ---------------- attention block (u, h, ih): 16 j-steps ----------
        def attention_block(u, h, ih, fill, last=False):
            kTh = qk["kTA" if h == 0 else "kTB"]
            qTh = qk["qTA" if h == 0 else "qTB"]
            i0 = u * N + ih * IH
            cps = psum.tile([65, IH], mybir.dt.float32, name=f"cps{u}{h}{ih}",
                            tag="cps")
            sps = [None, None]
            pexp = [None, None]

            def scores(jt):
                j0 = u * N + jt * JT
                sp = psum.tile([128, IH], mybir.dt.float32,
                               name=f"sps{jt % 2}", tag=f"sps{jt % 2}")
                nc.tensor.matmul(sp[:, 0:512], kTh[0:64, j0:j0 + JT],
                                 qTh[0:64, i0:i0 + 512], start=True, stop=True)
                nc.tensor.matmul(sp[:, 512:1024], kTh[64:128, j0:j0 + JT],
                                 qTh[64:128, i0 + 512:i0 + 1024],
                                 start=True, stop=True)
                sps[jt % 2] = sp

            def expstep(jt):
                pe = ppool.tile([128, IH], DT, tag=f"pexp{jt % 2}")
                nc.scalar.activation(pe[:], sps[jt % 2][:], AF.Exp, scale=SCALE)
                pexp[jt % 2] = pe

            def ctxstep(jt):
                pe = pexp[jt % 2]
                st = dict(start=(jt == 0), stop=(jt == NJT - 1))
                va = v_aug[u][:, jt, 65 * h:65 * h + 65]
                nc.tensor.matmul(cps[:, 0:512], va, pe[:, 0:512], **st)
                nc.tensor.matmul(cps[:, 512:1024], va, pe[:, 512:1024], **st)

            scores(0)
            expstep(0)
            for jt in range(NJT):
                if jt + 1 < NJT:
                    scores(jt + 1)
                    expstep(jt + 1)
                ctxstep(jt)
                # no fills near the block boundary: lets the DVE queue
                # drain so the ctx_s eviction frees cps without delay
                if jt < NJT - 2:
                    fill()
            # normalize: ctxT rows 64h..64h+64 = cps[0:64] * (1/r).
            # Normally copy cps out to SBUF first so the psum accumulator
            # frees immediately (next block's ctx j-step 0 reuses it). The
            # final block instead reads cps directly in 512-col halves so
            # the tail out-proj can start as soon as half 0 lands.
            if not last:
                ctx_s = rpool.tile([65, IH], mybir.dt.float32, tag="ctx_s")
                nc.vector.tensor_copy(ctx_s[:], cps[:])
                src = ctx_s
            else:
                src = cps
            rinv = rpool.tile([1, IH], mybir.dt.float32, name="rinv", tag="rinv")
            rb = rpool.tile([64, IH], mybir.dt.float32, name="rb", tag="rb")
            for half in range(2):
                hs = slice(half * 512, (half + 1) * 512)
                nc.vector.reciprocal(rinv[:, hs], src[64:65, hs])
                nc.gpsimd.partition_broadcast(rb[:, hs], rinv[:, hs])
                nc.vector.tensor_mul(
                    ctxT[u][64 * h:64 * h + 64,
                            ih * IH + half * 512:ih * IH + half * 512 + 512],
                    src[0:64, hs], rb[:, hs])

        # ---------------- driver ------------------------------------------
        fill_q = []

        def fill():
            if fill_q:
                fill_q.pop(0)()

        # chunk 0: load wq + xq first so the q chain starts earliest, then
        # the rest of the weights; chunk-0 compute runs inline. Chunk DMAs
        # are kicked one block ahead of their compute units.
        nc.sync.dma_start(bias_sb[:], aps["bias"])
        dma1, units1 = pass0_units(1)
        dma2, units2 = pass0_units(2)
        dma3, units3 = pass0_units(3)
        dma0, units0 = pass0_units(0)
        dma0()
        dma1()
        for unit in units0:
            unit()
        fill_q.extend(units1)

        blocks = [(0, 0, 0), (0, 1, 0), (0, 0, 1), (0, 1, 1),
                  (1, 0, 0), (1, 1, 0), (1, 0, 1), (1, 1, 1)]
        for bi, (u, h, ih) in enumerate(blocks):
            if bi == 1:
                dma2()
                fill_q.extend(units2)
            elif bi == 2:
                dma3()
                fill_q.extend(units3)
            elif bi == 4:
                # hold back 3 u0 tiles so the final blocks stay fed
                fill_q.extend(outproj_unit(0, tt) for tt in range(13))
            elif bi == 6:
                fill_q.extend(outproj_unit(0, tt) for tt in range(13, 16))
                fill_q.extend(outproj_unit(1, tt) for tt in range(8))
            attention_block(u, h, ih, fill, last=(bi == 7))
        while fill_q:
            fill_q.pop(0)()
        # tail out-proj: attention psum banks are free now — use the big
        # [128,1024] slots, and alternate evictions between the now-idle
        # ACT engine and DVE so consecutive tiles fully pipeline
        for i, tt in enumerate(range(8, N // 128)):
            ops = psum.tile([128, IH], mybir.dt.float32, name=f"ot{tt}",
                            tag=f"sps{i % 2}")
            for oc in range(2):
                nc.tensor.matmul(
                    ops[:, oc * 512:(oc + 1) * 512],
                    ctxT[1][:, tt * 128:(tt + 1) * 128],
                    wo_sb[:, oc * 512:(oc + 1) * 512],
                    start=True, stop=True)
            osb = opool.tile([128, C], DT, tag="osb")
            if i % 2 == 0:
                nc.scalar.copy(osb[:], ops[:])
            else:
                nc.vector.tensor_copy(osb[:], ops[:])
            nc.sync.dma_start(
                aps["out"][N + tt * 128:N + (tt + 1) * 128, :], osb[:])


def build():
    nc = bacc.Bacc("TRN2", target_bir_lowering=False, debug=False)
    aps = {
        "xq": nc.dram_tensor("xq", [C, T], DT, kind="ExternalInput").ap(),
        "xk": nc.dram_tensor("xk", [C, T], DT, kind="ExternalInput").ap(),
        "xv": nc.dram_tensor("xv", [C, T], DT, kind="ExternalInput").ap(),
        "wq": nc.dram_tensor("wq", [C, CH], DT, kind="ExternalInput").ap(),
        "wk": nc.dram_tensor("wk", [C, CH], DT, kind="ExternalInput").ap(),
        "wv": nc.dram_tensor("wv", [C, CH], DT, kind="ExternalInput").ap(),
        "wo": nc.dram_tensor("wo", [CH, C], DT, kind="ExternalInput").ap(),
        "bias": nc.dram_tensor("bias", [CH, 2], mybir.dt.float32, kind="ExternalInput").ap(),
        "out": nc.dram_tensor("out", [T, C], DT, kind="ExternalOutput").ap(),
    }
    with tile.TileContext(nc) as tc:
        emit(tc, aps)
    nc.compile()
    return nc


_NC = None


def make_in_maps(query, key, value, Wq, bq, Wk, bk, Wv, bv, Wo, bo):
    query, key, value, Wq, bq, Wk, bk, Wv, bv, Wo, bo = (
        np.asarray(a, dtype=np.float32)
        for a in (query, key, value, Wq, bq, Wk, bk, Wv, bv, Wo, bo)
    )
    xq = np.ascontiguousarray(query.reshape(T, C).T).astype(NPDT)
    xk = np.ascontiguousarray(key.reshape(T, C).T).astype(NPDT)
    xv = np.ascontiguousarray(value.reshape(T, C).T).astype(NPDT)
    in_maps = []
    for c in range(NCORES):
        r = slice(CH * c, CH * (c + 1))
        in_maps.append({
            "xq": xq, "xk": xk, "xv": xv,
            "wq": np.ascontiguousarray(Wq[r, :].T).astype(NPDT),
            "wk": np.ascontiguousarray(Wk[r, :].T).astype(NPDT),
            "wv": np.ascontiguousarray(Wv[r, :].T).astype(NPDT),
            "wo": np.ascontiguousarray(Wo[:, r].T).astype(NPDT),
            "bias": np.ascontiguousarray(
                np.stack([bq[r], bk[r]], axis=1).astype(np.float32)),
        })
    return in_maps


def finish(partials, Wv_bias_args):
    Wo, bv, bo = Wv_bias_args
    out = np.zeros((T, C), np.float64)
    for p in partials:
        out += p.astype(np.float64)
    out += (np.asarray(Wo, np.float64) @ np.asarray(bv, np.float64)) + np.asarray(bo, np.float64)
    return out.astype(np.float32).reshape(B, N, C)


def kernel(query, key, value, Wq, bq, Wk, bk, Wv, bv, Wo, bo,
           _trace=False, _return_results=False):
    global _NC
    if _NC is None:
        _NC = build()
    in_maps = make_in_maps(query, key, value, Wq, bq, Wk, bk, Wv, bv, Wo, bo)
    res = run_bass_kernel_spmd(_NC, in_maps, core_ids=list(range(NCORES)), trace=_trace)
    out = finish([r["out"] for r in res.results], (Wo, bv, bo))
    if _return_results:
        return out, res
    return out



# revision 10
# speedup vs baseline: 1.1488x; 1.1488x over previous
"""CrossAttention kernel for 8 TRN2 NeuronCores.

Sharding (Megatron head-parallel): core c owns heads {2c, 2c+1} = output
channels [128c, 128c+128).
  - column-parallel q/k/v projections (full activations in, per-core head
    channels out)
  - full attention for the core's heads (both batch elements)
  - row-parallel out projection -> partial [4096, 1024] fp32; host sums the
    8 partials. v-bias and out-bias fold exactly into the host-side
    epilogue: out = sum_c partial_c + Wo @ bv + bo (softmax rows sum to 1).

Per-core dataflow (t = b*2048 + n, 4096 tokens):
  pass0 (per 1024-token chunk): pj [128ch, 512] = W @ xT chains, evicted in
    ONE [128,512] tensor_scalar_add to qT/kT [128ch, T] (rows 0:64 = head A
    d-dims, 64:128 = head B -- the natural layout, no duplication).
    v [t, ch] chains -> v_aug[u] [128 j, 16 jt, 130] (cols 65h+64 are ones).
  attention block (u, iw in 0..3, BOTH heads, i-window 512): 16 j-steps:
    scores: sp[:, 0:512]   = kT[0:64, j].T   @ qT[0:64, iw]    (head A)
            sp[:, 512:1024]= kT[64:128, j].T @ qT[64:128, iw]  (head B)
            -- the two matmuls use disjoint PE row groups, run concurrently
    exp:    pexp [128, 1024] = exp(0.125 * sp)   (ACT, PSUM->SBUF fp16)
    ctx:    cps[:, 0:512]   += v_aug[., jt, 0:65].T   @ pexp[:, 0:512]
            cps[:, 512:1024]+= v_aug[., jt, 65:130].T @ pexp[:, 512:1024]
            (row 64 of each half accumulates the softmax denominator r[i])
  normalize: one DVE copy cps->ctx_s [65,1024]; reciprocal_approx_fast on
    row 64; gpsimd partition_broadcast; 2 DVE muls -> ctxT[u] fp16.
  outproj: po [128t, 512] = ctxT[u][:, tt].T @ wo half; evict fp16, DMA out.

Projection / out-proj work units are interleaved between attention j-steps
so the PE fills the slack under the ACT-bound softmax exp stream.
PSUM banks: sps0/sps1 [128,1024] (2+2), cps [65,1024] (2), pj/po [128,512]
(1+1) = 8.
"""

import numpy as np
from contextlib import ExitStack

import concourse.bass as bass
import concourse.tile as tile
from concourse import bacc, mybir
from concourse.bass_utils import run_bass_kernel_spmd

AF = mybir.ActivationFunctionType

# ---- problem constants (hardcoded per contract) ----
B, N, C, H, D = 2, 2048, 1024, 16, 64
T = B * N            # 4096 flattened tokens
CH = 128             # channels per core = 2 heads * 64
NCORES = 8
SCALE = D ** -0.5    # 0.125

# ---- tunables ----
DT = mybir.dt.float16      # on-chip matmul/storage dtype
NPDT = np.float16
TCH = 1024                 # pass0 t-chunk
NTCH = T // TCH            # 4
FT = C // 128              # 8 f-tiles (contraction tiles for projections)
IW = 512                   # attention i-window per block
JT = 128                   # j tile (kv) size
NJT = N // JT              # 16


def emit(tc: tile.TileContext, aps: dict):
    nc = tc.nc
    ctx = ExitStack()
    with ctx:
        const = ctx.enter_context(tc.tile_pool(name="const", bufs=1))
        persist = ctx.enter_context(tc.tile_pool(name="persist", bufs=1))
        xpool = ctx.enter_context(tc.tile_pool(name="xpool", bufs=2))
        ppool = ctx.enter_context(tc.tile_pool(name="ppool", bufs=2))
        rpool = ctx.enter_context(tc.tile_pool(name="rpool", bufs=2))
        opool = ctx.enter_context(tc.tile_pool(name="opool", bufs=4))
        psum = ctx.enter_context(tc.tile_pool(name="psum", bufs=1, space="PSUM"))

        # ---- load weights (wq first: the q chain of chunk 0 runs first) ----
        w_sb = {name: const.tile([128, FT, CH], DT, name=name, tag=name)
                for name in ("wq", "wk", "wv")}
        wo_sb = const.tile([128, C], DT, tag="wo")
        bias_sb = const.tile([128, 2], mybir.dt.float32, tag="bias")

        def load_weights(names):
            for name in names:
                nc.sync.dma_start(w_sb[name][:],
                                  aps[name].rearrange("(f p) m -> p f m", p=128))

        # ---- persistent activations ----
        # qT/kT [128 ch, T]: rows 0:64 head A dims, rows 64:128 head B
        qT = persist.tile([128, T], DT, name="qT", tag="qT")
        kT = persist.tile([128, T], DT, name="kT", tag="kT")
        # v_aug[u]: [128 j, 16 jt, 130]; cols 65h+64 are ones
        v_aug = [persist.tile([128, NJT, 130], DT, name=f"vaug{u}", tag=f"vaug{u}")
                 for u in range(B)]
        for u in range(B):
            for h in range(2):
                nc.vector.memset(v_aug[u][:, :, 65 * h + 64:65 * h + 65], 1.0)
        # ctxT[u]: [128 ch, 2048 i] normalized context^T (head B rows 64-127)
        ctxT = [persist.tile([128, N], DT, name=f"ctxT{u}", tag=f"ctxT{u}")
                for u in range(B)]

        xr = {k: aps[k].rearrange("(f p) t -> p f t", p=128) for k in ("xq", "xk", "xv")}

        # pass0 / outproj share the two 1-bank psum slots pj & po; pass0 is
        # done (chunk 3 consumed during block 3) before outproj starts
        # (block 4), so phases never contend.
        pingpong = [0]

        def small_ps(name):
            tag = ("pj", "po")[pingpong[0] % 2]
            pingpong[0] += 1
            return psum.tile([128, 512], mybir.dt.float32, name=name, tag=tag)

        # ---------------- pass0: projections for one 1024-token chunk -----
        # returns (dma_fn, [compute units]); the driver kicks the DMA a
        # block ahead of when the compute units get filled in.
        def pass0_units(tch):
            gen = []
            ts = slice(tch * TCH, (tch + 1) * TCH)
            xq_t = xpool.tile([128, FT, TCH], DT, tag="xq")
            xk_t = xpool.tile([128, FT, TCH], DT, tag="xk")
            xv_t = xpool.tile([128, FT, TCH], DT, tag="xv")

            order = ([("xq", xq_t), ("xk", xk_t), ("xv", xv_t)] if tch == 0
                     else [("xk", xk_t), ("xv", xv_t), ("xq", xq_t)])

            def u_dma():
                for nm, t in order:
                    if tch == 0:
                        # interleave weight loads so each chunk-0 chain's
                        # weight lands just before its activations
                        load_weights({"xq": ["wq"], "xk": ["wk"],
                                      "xv": ["wv"]}[nm])
                    nc.sync.dma_start(t[:], xr[nm][:, :, ts])
                if tch == 0:
                    nc.sync.dma_start(wo_sb[:], aps["wo"])

            def qk_half(x_t, wname, bcol, dst, half):
                # two sub-units sharing one psum accumulator: finer fill
                # granularity keeps per-jt PE slack small
                hs = slice(half * 512, (half + 1) * 512)
                gts = slice(tch * TCH + half * 512,
                            tch * TCH + half * 512 + 512)
                state = {}

                def unit_a():
                    pj = small_ps(f"c{tch}{wname}{half}")
                    state["pj"] = pj
                    for ft in range(4):
                        nc.tensor.matmul(pj[:], w_sb[wname][:, ft],
                                         x_t[:, ft, hs],
                                         start=(ft == 0), stop=False)

                def unit_b():
                    pj = state["pj"]
                    for ft in range(4, FT):
                        nc.tensor.matmul(pj[:], w_sb[wname][:, ft],
                                         x_t[:, ft, hs],
                                         start=False, stop=(ft == FT - 1))
                    nc.vector.tensor_scalar_add(
                        dst[:, gts], pj[:], bias_sb[:, bcol:bcol + 1])
                return [unit_a, unit_b]

            def v_half(vg, t4pair):
                # one 512-col psum with two t4 sub-chains (t4pair=0 -> t4 0,1)
                def unit():
                    pv = small_ps(f"c{tch}v{vg}{t4pair}")
                    pv4 = pv[:, 0:256].rearrange("p (t4 hh d) -> p t4 hh d",
                                                 t4=2, d=64)
                    for t4i in range(2):
                        t4 = t4pair * 2 + t4i
                        cs = slice(t4i * 128, (t4i + 1) * 128)
                        gcs = slice(vg * 512 + t4 * 128, vg * 512 + t4 * 128 + 128)
                        for ft in range(FT):
                            nc.tensor.matmul(pv[:, cs], xv_t[:, ft, gcs],
                                             w_sb["wv"][:, ft],
                                             start=(ft == 0), stop=(ft == FT - 1))
                    tt0 = tch * 8 + vg * 4 + t4pair * 2
                    u, jt0 = tt0 // 16, tt0 % 16
                    for h in range(2):
                        nc.vector.tensor_copy(
                            v_aug[u][:, jt0:jt0 + 2, 65 * h:65 * h + 64],
                            pv4[:, :, h])
                return unit

            q_units = [un for hf in range(2)
                       for un in qk_half(xq_t, "wq", 0, qT, hf)]
            k_units = [un for hf in range(2)
                       for un in qk_half(xk_t, "wk", 1, kT, hf)]
            v_units = [v_half(vg, tp) for vg in range(2) for tp in range(2)]
            if tch == 0:
                gen.extend(q_units + k_units + v_units)
            else:
                # k/v first: later j-steps of in-flight blocks need them
                gen.extend(k_units + v_units + q_units)
            return u_dma, gen

        # ---------------- out-proj for one 128-token tile ------------------
        oev = [0]

        def outproj_unit(u, tt):
            def unit():
                osb = opool.tile([128, C], DT, tag="osb")
                for oc in range(2):
                    po = small_ps(f"o{u}t{tt}{oc}")
                    nc.tensor.matmul(
                        po[:], ctxT[u][:, tt * 128:(tt + 1) * 128],
                        wo_sb[:, oc * 512:(oc + 1) * 512],
                        start=True, stop=True)
                    nc.vector.tensor_copy(osb[:, oc * 512:(oc + 1) * 512],
                                          po[:])
                    oev[0] += 1
                nc.sync.dma_start(
                    aps["out"][u * N + tt * 128:u * N + (tt + 1) * 128, :],
                    osb[:])
            return unit

        # ------------- attention block (u, iw): both heads, 16 j-steps ----
        def attention_block(u, iw, fill):
            i0 = u * N + iw * IW
            cps = psum.tile([65, 1024], mybir.dt.float32, name=f"cps{u}{iw}",
                            tag="cps")
            sps = [None, None]
            pexp = [None, None]

            def scores(jt):
                j0 = u * N + jt * JT
                sp = psum.tile([128, 1024], mybir.dt.float32,
                               name=f"sps{jt % 2}", tag=f"sps{jt % 2}")
                nc.tensor.matmul(sp[:, 0:512], kT[0:64, j0:j0 + JT],
                                 qT[0:64, i0:i0 + IW], start=True, stop=True)
                nc.tensor.matmul(sp[:, 512:1024], kT[64:128, j0:j0 + JT],
                                 qT[64:128, i0:i0 + IW], start=True, stop=True)
                sps[jt % 2] = sp

            def expstep(jt):
                pe = ppool.tile([128, 1024], DT, tag=f"pexp{jt % 2}")
                nc.scalar.activation(pe[:], sps[jt % 2][:], AF.Exp, scale=SCALE)
                pexp[jt % 2] = pe

            def ctxstep(jt):
                pe = pexp[jt % 2]
                st = dict(start=(jt == 0), stop=(jt == NJT - 1))
                nc.tensor.matmul(cps[:, 0:512], v_aug[u][:, jt, 0:65],
                                 pe[:, 0:512], **st)
                nc.tensor.matmul(cps[:, 512:1024], v_aug[u][:, jt, 65:130],
                                 pe[:, 512:1024], **st)

            scores(0)
            expstep(0)
            for jt in range(NJT):
                if jt + 1 < NJT:
                    scores(jt + 1)
                    expstep(jt + 1)
                ctxstep(jt)
                # no fills near the block boundary: lets the DVE queue
                # drain so the ctx_s eviction frees cps without delay
                if jt < NJT - 2:
                    fill()
            # normalize: pull the denominator row to a partition-0 tile with
            # a native ACT copy (custom-ucode ops ignore input partition
            # offsets on HW!), copy the ctx rows out on DVE so the psum
            # accumulator frees, then 1/r (fast approx), broadcast, scale.
            rrow = rpool.tile([1, 1024], mybir.dt.float32, name="rrow",
                              tag="rrow")
            nc.scalar.copy(rrow[:], cps[64:65, :])
            ctx_s = rpool.tile([64, 1024], mybir.dt.float32, tag="ctx_s")
            nc.vector.tensor_copy(ctx_s[:], cps[0:64, :])
            rinv = rpool.tile([1, 1024], mybir.dt.float32, name="rinv",
                              tag="rinv")
            nc.vector.reciprocal_approx_fast(out=rinv[:], in_=rrow[:])
            rb = rpool.tile([64, 1024], mybir.dt.float32, name="rb", tag="rb")
            nc.gpsimd.partition_broadcast(rb[:], rinv[:])
            for h in range(2):
                nc.vector.tensor_mul(
                    ctxT[u][64 * h:64 * h + 64, iw * IW:(iw + 1) * IW],
                    ctx_s[:, 512 * h:512 * h + 512],
                    rb[:, 512 * h:512 * h + 512])

        # ---------------- driver ------------------------------------------
        fill_q = []

        def fill():
            if fill_q:
                fill_q.pop(0)()

        # chunk 0: load wq + xq first so the q chain starts earliest, then
        # the rest of the weights; chunk-0 compute runs inline. Chunk DMAs
        # are kicked one block ahead of their compute units.
        nc.sync.dma_start(bias_sb[:], aps["bias"])
        dma1, units1 = pass0_units(1)
        dma2, units2 = pass0_units(2)
        dma3, units3 = pass0_units(3)
        dma0, units0 = pass0_units(0)
        dma0()
        dma1()
        for unit in units0:
            unit()
        fill_q.extend(units1)

        blocks = [(0, 0), (0, 1), (0, 2), (0, 3),
                  (1, 0), (1, 1), (1, 2), (1, 3)]
        for bi, (u, iw) in enumerate(blocks):
            if bi == 1:
                dma2()
                fill_q.extend(units2)
            elif bi == 2:
                dma3()
                fill_q.extend(units3)
            elif bi == 4:
                # u0 fully normalized after block 3
                fill_q.extend(outproj_unit(0, tt) for tt in range(8))
            elif bi == 5:
                fill_q.extend(outproj_unit(0, tt) for tt in range(8, 16))
            elif bi == 6:
                # u1 tokens 0:1024 normalized after block 5
                fill_q.extend(outproj_unit(1, tt) for tt in range(4))
            elif bi == 7:
                fill_q.extend(outproj_unit(1, tt) for tt in range(4, 8))
            attention_block(u, iw, fill)
        while fill_q:
            fill_q.pop(0)()
        # tail out-proj: attention psum banks are free now — use the big
        # [128,1024] slots, and alternate evictions between the now-idle
        # ACT engine and DVE so consecutive tiles fully pipeline
        for i, tt in enumerate(range(8, N // 128)):
            ops = psum.tile([128, 1024], mybir.dt.float32, name=f"ot{tt}",
                            tag=f"sps{i % 2}")
            for oc in range(2):
                nc.tensor.matmul(
                    ops[:, oc * 512:(oc + 1) * 512],
                    ctxT[1][:, tt * 128:(tt + 1) * 128],
                    wo_sb[:, oc * 512:(oc + 1) * 512],
                    start=True, stop=True)
            osb = opool.tile([128, C], DT, tag="osb")
            if i % 2 == 0:
                nc.scalar.copy(osb[:], ops[:])
            else:
                nc.vector.tensor_copy(osb[:], ops[:])
            nc.sync.dma_start(
                aps["out"][N + tt * 128:N + (tt + 1) * 128, :], osb[:])


def build():
    nc = bacc.Bacc("TRN2", target_bir_lowering=False, debug=False)
    aps = {
        "xq": nc.dram_tensor("xq", [C, T], DT, kind="ExternalInput").ap(),
        "xk": nc.dram_tensor("xk", [C, T], DT, kind="ExternalInput").ap(),
        "xv": nc.dram_tensor("xv", [C, T], DT, kind="ExternalInput").ap(),
        "wq": nc.dram_tensor("wq", [C, CH], DT, kind="ExternalInput").ap(),
        "wk": nc.dram_tensor("wk", [C, CH], DT, kind="ExternalInput").ap(),
        "wv": nc.dram_tensor("wv", [C, CH], DT, kind="ExternalInput").ap(),
        "wo": nc.dram_tensor("wo", [CH, C], DT, kind="ExternalInput").ap(),
        "bias": nc.dram_tensor("bias", [CH, 2], mybir.dt.float32, kind="ExternalInput").ap(),
        "out": nc.dram_tensor("out", [T, C], DT, kind="ExternalOutput").ap(),
    }
    with tile.TileContext(nc) as tc:
        emit(tc, aps)
    nc.compile()
    return nc


_NC = None


def make_in_maps(query, key, value, Wq, bq, Wk, bk, Wv, bv, Wo, bo):
    query, key, value, Wq, bq, Wk, bk, Wv, bv, Wo, bo = (
        np.asarray(a, dtype=np.float32)
        for a in (query, key, value, Wq, bq, Wk, bk, Wv, bv, Wo, bo)
    )
    xq = np.ascontiguousarray(query.reshape(T, C).T).astype(NPDT)
    xk = np.ascontiguousarray(key.reshape(T, C).T).astype(NPDT)
    xv = np.ascontiguousarray(value.reshape(T, C).T).astype(NPDT)
    in_maps = []
    for c in range(NCORES):
        r = slice(CH * c, CH * (c + 1))
        in_maps.append({
            "xq": xq, "xk": xk, "xv": xv,
            "wq": np.ascontiguousarray(Wq[r, :].T).astype(NPDT),
            "wk": np.ascontiguousarray(Wk[r, :].T).astype(NPDT),
            "wv": np.ascontiguousarray(Wv[r, :].T).astype(NPDT),
            "wo": np.ascontiguousarray(Wo[:, r].T).astype(NPDT),
            "bias": np.ascontiguousarray(
                np.stack([bq[r], bk[r]], axis=1).astype(np.float32)),
        })
    return in_maps


def finish(partials, Wv_bias_args):
    Wo, bv, bo = Wv_bias_args
    out = np.zeros((T, C), np.float64)
    for p in partials:
        out += p.astype(np.float64)
    out += (np.asarray(Wo, np.float64) @ np.asarray(bv, np.float64)) + np.asarray(bo, np.float64)
    return out.astype(np.float32).reshape(B, N, C)


def kernel(query, key, value, Wq, bq, Wk, bk, Wv, bv, Wo, bo,
           _trace=False, _return_results=False):
    global _NC
    if _NC is None:
        _NC = build()
    in_maps = make_in_maps(query, key, value, Wq, bq, Wk, bk, Wv, bv, Wo, bo)
    res = run_bass_kernel_spmd(_NC, in_maps, core_ids=list(range(NCORES)), trace=_trace)
    out = finish([r["out"] for r in res.results], (Wo, bv, bo))
    if _return_results:
        return out, res
    return out
